# revision 2
# baseline (speedup 1.0000x reference)
"""Trainium2 kernel for nn_EuclideanEmbedding (edge-scale + segment_sum).

Computes: out[n, :] = inv * sum_{e: receivers[e]==n} sh_vectors[e, :] * cutoffs[e]

Distribution (host side, inside kernel()):
  - Edges sharded across the 8 NeuronCores BY RECEIVER NODE RANGE: core c owns
    nodes [c*6250, (c+1)*6250) and exactly the edges targeting them.  Each core
    produces a disjoint output slice, so no collective is needed.

Device layout (v2 — bf16 + PE-array segmented reduction):
  - Nodes are degree-sorted (desc) and packed into capacity groups.  A group
    with slot capacity c stacks k = floor(128/c) node-columns on the partition
    axis: partition p = n_lo*c + s  (n_lo in [0,k), slot s in [0,c)).
    Free axis per node-column block of 8 nodes: col = 128*i + d*8 + n8.
  - sh and cutoffs are converted to bf16 on the host (harness gate is
    rel_err < 2e-2; bf16 keeps us ~5e-3).  This halves HBM traffic — the
    fp32 baseline was HBM-bound at ~64us of DMA window.
  - The elementwise multiply (sh * cut) runs as bf16 TensorTensor ops split
    between the Vector and GpSimd engines (bf16 packed innermost -> DVE 2x
    mode).  The cut operand broadcasts over d via a zero-stride AP dim.
  - The segmented reduction runs on the OTHERWISE-IDLE PE array: one matmul
    per 8-node block with lhsT (stationary) = scaled data [kc, 128] and
    rhs (moving) = a static ones-block [kc, k] (ones[p, j] = (p // c == j)).
    out[d*8+n8, j] = sum_s scl[(j,s), d*8+n8] lands in PSUM fp32.  This
    removes the ~30us dtype-independent DVE tensor_reduce of the baseline.
  - PSUM eviction via the Scalar (ACT) engine activation-copy, which also
    applies the inv_avg_num_neighbors scale for free.
  - Output written per chunk (overlapped), not as one serial tail DMA.
"""

import os

import ml_dtypes
import numpy as np

# ---------------------------------------------------------------- constants
N_NODES = 50_000
D_SH = 16
N_CORES = 8
NPC = N_NODES // N_CORES          # 6250 nodes per core
NB_MAX = 28                       # max 8-node blocks per chunk (DMA/psum unit)
V_FRAC = 0.78                     # fraction of multiply blocks on Vector (rest GpSimd)

_NC_CACHE: dict = {}
LAST_RESULTS = None  # BassKernelResults of the most recent run (for test.py)


# ---------------------------------------------------------------- geometry
def plan_chunks(dmax: np.ndarray):
    """Greedy capacity grouping from the SPMD-uniform per-position max degree
    (descending).  Returns a list of chunk dicts with all offsets."""
    q, groups = 0, []
    npos = dmax.shape[0]
    while q < npos:
        d0 = max(int(dmax[q]), 1)
        assert d0 <= 128, f"node degree {d0} > 128 unsupported by this layout"
        k = max(1, 128 // d0)
        q2 = q
        while q2 < npos and max(1, 128 // max(int(dmax[q2]), 1)) == k:
            q2 += 1
        n = -(-(q2 - q) // (8 * k)) * (8 * k)
        n = min(n, -(-(npos - q) // (8 * k)) * (8 * k))
        c = int(dmax[q:min(q + n, npos)].max())
        groups.append((k, max(c, 1), n))
        q += n

    chunks = []
    node_start = sh_off = cut_off = out_off = ones_off = 0
    for k, c, n in groups:
        nb_total = n // (8 * k)
        done = 0
        while done < nb_total:
            nb = min(NB_MAX, nb_total - done)
            kc = k * c
            F = 128 * nb
            nbv = max(0, min(nb, int(round(nb * V_FRAC))))
            chunks.append(dict(k=k, c=c, kc=kc, nb=nb, nbv=nbv, F=F,
                               node_start=node_start, sh_off=sh_off,
                               cut_off=cut_off, out_off=out_off,
                               ones_off=ones_off))
            node_start += 8 * k * nb
            sh_off += kc * F
            cut_off += 8 * nb
            out_off += k * nb
            ones_off += k
            done += nb
    tot = dict(nodes=node_start, sh=sh_off, cut=cut_off, out=out_off,
               ones=ones_off)
    return chunks, tot


# ---------------------------------------------------------------- device IR
def build_nc(key, chunks, tot):
    if key in _NC_CACHE:
        return _NC_CACHE[key]

    import concourse.bacc as bacc
    import concourse.bass as bass
    import concourse.mybir as mybir
    from concourse import tile

    nc = bacc.Bacc("TRN2", target_bir_lowering=False, debug=False)

    sh = nc.dram_tensor("sh", [tot["sh"]], mybir.dt.bfloat16,
                        kind="ExternalInput")
    cut = nc.dram_tensor("cut", [128, tot["cut"]], mybir.dt.bfloat16,
                         kind="ExternalInput")
    ones = nc.dram_tensor("ones", [128, tot["ones"]], mybir.dt.bfloat16,
                          kind="ExternalInput")
    inv = nc.dram_tensor("inv", [128, 1], mybir.dt.float32,
                         kind="ExternalInput")
    out = nc.dram_tensor("out", [128, tot["out"]], mybir.dt.float32,
                         kind="ExternalOutput")

    with tile.TileContext(nc) as tc:
        with (
            tc.tile_pool(name="const", bufs=1) as cpool,
            tc.tile_pool(name="shp", bufs=3) as shp,
            tc.tile_pool(name="sclv", bufs=2) as sclvp,
            tc.tile_pool(name="sclp", bufs=2) as sclpp,
            tc.tile_pool(name="osb", bufs=2) as osbp,
            tc.tile_pool(name="ps", bufs=2, space="PSUM") as psp,
        ):
            inv_t = cpool.tile([128, 1], mybir.dt.float32)
            nc.sync.dma_start(inv_t[:], inv[:])
            cut_t = cpool.tile([128, tot["cut"]], mybir.dt.bfloat16)
            nc.sync.dma_start(cut_t[:], cut[:])
            ones_t = cpool.tile([128, tot["ones"]], mybir.dt.bfloat16)
            nc.sync.dma_start(ones_t[:], ones[:])

            for ch in chunks:
                k, c, kc, nb, nbv, F = (ch["k"], ch["c"], ch["kc"], ch["nb"],
                                        ch["nbv"], ch["F"])
                sh_t = shp.tile([kc, F], mybir.dt.bfloat16, tag="sh")
                src = bass.AP(sh.ap().tensor, ch["sh_off"], [[F, kc], [1, F]])
                nc.sync.dma_start(sh_t[:], src)

                # scl[p, i, d, n8] = sh[p, i, d, n8] * cut[p, 8*i + n8]
                # (split by block range between Vector and GpSimd)
                halves = []
                if nbv > 0:
                    halves.append((nc.vector, 0, nbv, sclvp, "sclv"))
                if nbv < nb:
                    halves.append((nc.gpsimd, nbv, nb, sclpp, "sclp"))
                scl_tiles = {}
                for eng, b0, b1, pool, tg in halves:
                    nbe = b1 - b0
                    scl = pool.tile([kc, nbe * 128], mybir.dt.bfloat16, tag=tg)
                    pdim = list(sh_t[:].ap[0])
                    sh4 = bass.AP(sh_t[:].tensor, sh_t[:].offset + b0 * 128,
                                  [pdim, [128, nbe], [8, D_SH], [1, 8]])
                    scl4 = bass.AP(scl[:].tensor, scl[:].offset,
                                   [list(scl[:].ap[0]), [128, nbe],
                                    [8, D_SH], [1, 8]])
                    cut4 = bass.AP(cut_t[:].tensor,
                                   cut_t[:].offset + ch["cut_off"] + b0 * 8,
                                   [[cut_t[:].ap[0][0], kc], [8, nbe],
                                    [0, D_SH], [1, 8]])
                    eng.tensor_mul(scl4, sh4, cut4)
                    scl_tiles[tg] = (scl, b0, b1)

                # PE: per 8-node block, out[(d,n8), j] = sum_s scl[(j,s),...]
                ps_t = psp.tile([128, k * nb], mybir.dt.float32, tag="ps")
                ones_ap = bass.AP(ones_t[:].tensor,
                                  ones_t[:].offset + ch["ones_off"],
                                  [[ones_t[:].ap[0][0], kc], [1, k]])
                for tg, (scl, b0, b1) in scl_tiles.items():
                    for i in range(b0, b1):
                        lhsT = bass.AP(scl[:].tensor,
                                       scl[:].offset + (i - b0) * 128,
                                       [list(scl[:].ap[0]), [1, 128]])
                        nc.tensor.matmul(ps_t[:, i * k:(i + 1) * k],
                                         lhsT, ones_ap)

                # evict PSUM -> SBUF with the inv scale folded in, then out
                osb = osbp.tile([128, k * nb], mybir.dt.float32, tag="osb")
                nc.scalar.mul(osb[:], ps_t[:], inv_t[:])
                dst = bass.AP(out.ap().tensor, ch["out_off"],
                              [[tot["out"], 128], [1, k * nb]])
                nc.sync.dma_start(dst, osb[:])

    nc.compile()
    _NC_CACHE[key] = nc
    return nc


# ---------------------------------------------------------------- host shard
def shard_inputs(sh_vectors, cutoffs, receivers, inv_avg_num_neighbors):
    sh_np = np.ascontiguousarray(np.asarray(sh_vectors, dtype=np.float32))
    cut_np = np.asarray(cutoffs, dtype=np.float32).ravel()
    rec = np.asarray(receivers).astype(np.int64).ravel()
    inv_val = np.float32(np.asarray(inv_avg_num_neighbors).ravel()[0])

    order = np.argsort(rec, kind="stable")
    rec_sorted = rec[order]
    first = np.searchsorted(rec_sorted, rec_sorted, side="left")
    occ = (np.arange(rec.size) - first).astype(np.int64)
    bounds = np.searchsorted(rec_sorted, np.arange(0, N_NODES + 1, NPC))

    deg = np.bincount(rec, minlength=N_NODES).reshape(N_CORES, NPC)
    node_orders, pos_of_node, dsort = [], [], np.empty((N_CORES, NPC), np.int64)
    for c in range(N_CORES):
        no = np.argsort(-deg[c], kind="stable")
        node_orders.append(no)
        pon = np.empty(NPC, dtype=np.int64)
        pon[no] = np.arange(NPC)
        pos_of_node.append(pon)
        dsort[c] = deg[c][no]
    dmax = dsort.max(axis=0)

    chunks, tot = plan_chunks(dmax)

    # per-chunk lookup arrays indexed by chunk id
    nch = len(chunks)
    g_start = np.array([ch["node_start"] for ch in chunks], np.int64)
    g_k = np.array([ch["k"] for ch in chunks], np.int64)
    g_c = np.array([ch["c"] for ch in chunks], np.int64)
    g_F = np.array([ch["F"] for ch in chunks], np.int64)
    g_sh = np.array([ch["sh_off"] for ch in chunks], np.int64)
    g_cut = np.array([ch["cut_off"] for ch in chunks], np.int64)
    g_out = np.array([ch["out_off"] for ch in chunks], np.int64)
    g_end = np.concatenate([g_start[1:], [tot["nodes"]]])

    # static ones blocks (inv is NOT folded here; it rides the eviction)
    ones_dev = np.zeros((128, tot["ones"]), dtype=ml_dtypes.bfloat16)
    for ch in chunks:
        k, c = ch["k"], ch["c"]
        p = np.arange(k * c)
        ones_dev[p, ch["ones_off"] + p // c] = 1.0

    inv_dev = np.full((128, 1), inv_val, dtype=np.float32)

    sh_bf = sh_np.astype(ml_dtypes.bfloat16)
    cut_bf = cut_np.astype(ml_dtypes.bfloat16)

    in_maps = []
    decode = []   # per core arrays to invert the layout on output
    for cid in range(N_CORES):
        lo, hi = bounds[cid], bounds[cid + 1]
        e = order[lo:hi]
        l = rec_sorted[lo:hi] - cid * NPC
        o = occ[lo:hi]
        q = pos_of_node[cid][l]

        gi = np.searchsorted(g_start, q, side="right") - 1
        k_e, c_e, F_e = g_k[gi], g_c[gi], g_F[gi]
        dq = q - g_start[gi]
        jj = dq // k_e
        n_lo = dq - jj * k_e
        i_loc = jj >> 3
        n8 = jj & 7
        p = n_lo * c_e + o
        assert (o < c_e).all(), "slot overflow: degree exceeds group capacity"

        sh_dev = np.zeros(tot["sh"], dtype=ml_dtypes.bfloat16)
        base = g_sh[gi] + p * F_e + 128 * i_loc + n8
        shv = sh_bf[e]
        for d in range(D_SH):
            sh_dev[base + 8 * d] = shv[:, d]
        cut_dev = np.zeros((128, tot["cut"]), dtype=ml_dtypes.bfloat16)
        cut_dev[p, g_cut[gi] + jj] = cut_bf[e]
        in_maps.append({"sh": sh_dev, "cut": cut_dev, "ones": ones_dev,
                        "inv": inv_dev})

        # output decode for every position q in [0, NPC)
        qq = np.arange(NPC, dtype=np.int64)
        gq = np.searchsorted(g_start, qq, side="right") - 1
        dqq = qq - g_start[gq]
        jjq = dqq // g_k[gq]
        col = g_out[gq] + (jjq >> 3) * g_k[gq] + (dqq - jjq * g_k[gq])
        row8 = jjq & 7
        decode.append((col, row8))

    return in_maps, chunks, tot, node_orders, decode


# ---------------------------------------------------------------- profiling
def _install_ntff_shim() -> bool:
    """This image's antenv lacks the axon_hooks shim that bass_utils imports
    for trace=True under axon.  Recreate it from trn_agent_boot's ctypes hook
    so NTFF profiling works.  Returns True on success."""
    try:
        import sys
        import types

        import antenv

        if getattr(antenv, "axon_hooks", None) is not None:
            return True
        import trn_agent_boot.trn_boot as tb

        hook = tb._ntff_profile_via_ctypes("/opt/axon/libaxon_pjrt.so")
        mod = types.ModuleType("antenv.axon_hooks")
        mod._hook = hook
        mod.get_axon_ntff_profile_hook = lambda: mod._hook
        mod.set_axon_ntff_profile_hook = lambda h: setattr(mod, "_hook", h)
        sys.modules["antenv.axon_hooks"] = mod
        antenv.axon_hooks = mod
        return hook is not None
    except Exception as e:  # profiling is best-effort; the run must not break
        print(f"ntff shim unavailable: {e!r}")
        return False


# ---------------------------------------------------------------- entrypoint
def kernel(sh_vectors, cutoffs, receivers, inv_avg_num_neighbors) -> np.ndarray:
    global LAST_RESULTS
    from concourse.bass_utils import run_bass_kernel_spmd

    in_maps, chunks, tot, node_orders, decode = shard_inputs(
        sh_vectors, cutoffs, receivers, inv_avg_num_neighbors)
    key = tuple((ch["k"], ch["c"], ch["nb"]) for ch in chunks)
    nc = build_nc(key, chunks, tot)

    trace = os.environ.get("KERNEL_TRACE", "0") == "1"
    if trace:
        trace = _install_ntff_shim()
    res = run_bass_kernel_spmd(nc, in_maps, core_ids=list(range(N_CORES)),
                               trace=trace)
    LAST_RESULTS = res

    full = np.empty((N_NODES, D_SH), dtype=np.float32)
    for cid in range(N_CORES):
        o = np.asarray(res.results[cid]["out"], dtype=np.float32)
        col, row8 = decode[cid]
        blk = np.empty((NPC, D_SH), dtype=np.float32)
        for d in range(D_SH):
            blk[:, d] = o[8 * d + row8, col]
        full[cid * NPC + node_orders[cid]] = blk
    return full


# revision 12
# speedup vs baseline: 1.0815x; 1.0815x over previous
"""Trainium2 kernel for nn_EuclideanEmbedding (edge-scale + segment_sum).

Computes: out[n, :] = inv * sum_{e: receivers[e]==n} sh_vectors[e, :] * cutoffs[e]

Distribution (host side, inside kernel()):
  - Edges sharded across the 8 NeuronCores BY RECEIVER NODE RANGE: core c owns
    nodes [c*6250, (c+1)*6250) and exactly the edges targeting them.  Each core
    produces a disjoint output slice, so no collective is needed.

Device layout (v2 — bf16 + PE-array segmented reduction):
  - Nodes are degree-sorted (desc) and packed into capacity groups.  A group
    with slot capacity c stacks k = floor(128/c) node-columns on the partition
    axis: partition p = n_lo*c + s  (n_lo in [0,k), slot s in [0,c)).
    Free axis per node-column block of 8 nodes: col = 128*i + d*8 + n8.
  - sh and cutoffs are converted to bf16 on the host (harness gate is
    rel_err < 2e-2; bf16 keeps us ~5e-3).  This halves HBM traffic — the
    fp32 baseline was HBM-bound at ~64us of DMA window.
  - The elementwise multiply (sh * cut) runs as bf16 TensorTensor ops split
    between the Vector and GpSimd engines (bf16 packed innermost -> DVE 2x
    mode).  The cut operand broadcasts over d via a zero-stride AP dim.
  - The segmented reduction runs on the OTHERWISE-IDLE PE array: one matmul
    per 8-node block with lhsT (stationary) = scaled data [kc, 128] and
    rhs (moving) = a static ones-block [kc, k] (ones[p, j] = (p // c == j)).
    out[d*8+n8, j] = sum_s scl[(j,s), d*8+n8] lands in PSUM fp32.  This
    removes the ~30us dtype-independent DVE tensor_reduce of the baseline.
  - PSUM eviction via the Scalar (ACT) engine activation-copy, which also
    applies the inv_avg_num_neighbors scale for free.
  - Output written per chunk (overlapped), not as one serial tail DMA.
"""

import os

import ml_dtypes
import numpy as np

# ---------------------------------------------------------------- constants
N_NODES = 50_000
D_SH = 16
N_CORES = 8
NPC = N_NODES // N_CORES          # 6250 nodes per core
NB_MAX = 28                       # max 8-node blocks per chunk (DMA/psum unit)
V_FRAC = 0.875                    # fraction of multiply blocks on Vector (rest
                                  # GpSimd; Pool measured ~3.1 ns/col vs DVE
                                  # bf16-2x at ~0.47 ns/col)

_NC_CACHE: dict = {}
LAST_RESULTS = None  # BassKernelResults of the most recent run (for test.py)


# ---------------------------------------------------------------- geometry
def plan_chunks(dmax: np.ndarray):
    """Greedy capacity grouping from the SPMD-uniform per-position max degree
    (descending).  Returns a list of chunk dicts with all offsets."""
    q, groups = 0, []
    npos = dmax.shape[0]
    while q < npos:
        d0 = max(int(dmax[q]), 1)
        assert d0 <= 128, f"node degree {d0} > 128 unsupported by this layout"
        k = max(1, 128 // d0)
        q2 = q
        while q2 < npos and max(1, 128 // max(int(dmax[q2]), 1)) == k:
            q2 += 1
        n = -(-(q2 - q) // (8 * k)) * (8 * k)
        n = min(n, -(-(npos - q) // (8 * k)) * (8 * k))
        c = int(dmax[q:min(q + n, npos)].max())
        groups.append((k, max(c, 1), n))
        q += n

    chunks = []
    node_start = sh_off = cut_off = out_off = ones_off = 0
    for k, c, n in groups:
        nb_total = n // (8 * k)
        done = 0
        while done < nb_total:
            nb = min(NB_MAX, nb_total - done)
            kc = k * c
            F = 128 * nb
            nbv = max(0, min(nb, int(round(nb * V_FRAC))))
            chunks.append(dict(k=k, c=c, kc=kc, nb=nb, nbv=nbv, F=F,
                               node_start=node_start, sh_off=sh_off,
                               cut_off=cut_off, out_off=out_off,
                               ones_off=ones_off))
            node_start += 8 * k * nb
            sh_off += kc * F
            cut_off += 8 * nb
            out_off += k * nb
            ones_off += k
            done += nb
    tot = dict(nodes=node_start, sh=sh_off, cut=cut_off, out=out_off,
               ones=ones_off)
    # device-side processing order: smallest chunks first, so the first
    # multiply starts as soon as possible (pipeline priming); host layout
    # offsets above are order-independent.
    sched = sorted(range(len(chunks)), key=lambda i: chunks[i]["kc"] * chunks[i]["F"])
    return chunks, tot, sched


# ---------------------------------------------------------------- device IR
def build_nc(key, chunks, tot, sched):
    if key in _NC_CACHE:
        return _NC_CACHE[key]

    import concourse.bacc as bacc
    import concourse.bass as bass
    import concourse.mybir as mybir
    from concourse import tile

    nc = bacc.Bacc("TRN2", target_bir_lowering=False, debug=False)

    sh = nc.dram_tensor("sh", [tot["sh"]], mybir.dt.bfloat16,
                        kind="ExternalInput")
    cut = nc.dram_tensor("cut", [128, tot["cut"]], mybir.dt.bfloat16,
                         kind="ExternalInput")
    ones = nc.dram_tensor("ones", [128, tot["ones"]], mybir.dt.bfloat16,
                          kind="ExternalInput")
    out = nc.dram_tensor("out", [128, tot["out"]], mybir.dt.float32,
                         kind="ExternalOutput")

    with tile.TileContext(nc) as tc:
        with (
            tc.tile_pool(name="const", bufs=1) as cpool,
            tc.tile_pool(name="shp", bufs=4) as shp,
            tc.tile_pool(name="sclv", bufs=3) as sclvp,
            tc.tile_pool(name="sclp", bufs=3) as sclpp,
            tc.tile_pool(name="osb", bufs=2) as osbp,
            tc.tile_pool(name="ps", bufs=4, space="PSUM") as psp,
        ):
            cut_t = cpool.tile([128, tot["cut"]], mybir.dt.bfloat16)
            nc.sync.dma_start(cut_t[:], cut[:])
            ones_t = cpool.tile([128, tot["ones"]], mybir.dt.bfloat16)

            for chi, ci in enumerate(sched):
                ch = chunks[ci]
                k, c, kc, nb, nbv, F = (ch["k"], ch["c"], ch["kc"], ch["nb"],
                                        ch["nbv"], ch["F"])
                sh_t = shp.tile([kc, F], mybir.dt.bfloat16, tag="sh")
                src = bass.AP(sh.ap().tensor, ch["sh_off"], [[F, kc], [1, F]])
                nc.sync.dma_start(sh_t[:], src)
                if chi == 0:
                    # emit after the first sh chunk: ones is only needed by
                    # the first matmul, sh0+cut gate the first multiply
                    nc.sync.dma_start(ones_t[:], ones[:])

                # scl[p, i, d, n8] = sh[p, i, d, n8] * cut[p, 8*i + n8]
                # (split by block range between Vector and GpSimd)
                halves = []
                if nbv > 0:
                    halves.append((nc.vector, 0, nbv, sclvp, "sclv"))
                if nbv < nb:
                    halves.append((nc.gpsimd, nbv, nb, sclpp, "sclp"))
                scl_tiles = {}
                for eng, b0, b1, pool, tg in halves:
                    nbe = b1 - b0
                    scl = pool.tile([kc, nbe * 128], mybir.dt.bfloat16, tag=tg)
                    pdim = list(sh_t[:].ap[0])
                    sh4 = bass.AP(sh_t[:].tensor, sh_t[:].offset + b0 * 128,
                                  [pdim, [128, nbe], [8, D_SH], [1, 8]])
                    scl4 = bass.AP(scl[:].tensor, scl[:].offset,
                                   [list(scl[:].ap[0]), [128, nbe],
                                    [8, D_SH], [1, 8]])
                    cut4 = bass.AP(cut_t[:].tensor,
                                   cut_t[:].offset + ch["cut_off"] + b0 * 8,
                                   [[cut_t[:].ap[0][0], kc], [8, nbe],
                                    [0, D_SH], [1, 8]])
                    eng.tensor_mul(scl4, sh4, cut4)
                    scl_tiles[tg] = (scl, b0, b1)

                # PE: per 8-node block, out[(d,n8), j] = sum_s scl[(j,s),...]
                ps_t = psp.tile([128, k * nb], mybir.dt.float32, tag="ps")
                ones_ap = bass.AP(ones_t[:].tensor,
                                  ones_t[:].offset + ch["ones_off"],
                                  [[ones_t[:].ap[0][0], kc], [1, k]])
                for tg, (scl, b0, b1) in scl_tiles.items():
                    for i in range(b0, b1):
                        lhsT = bass.AP(scl[:].tensor,
                                       scl[:].offset + (i - b0) * 128,
                                       [list(scl[:].ap[0]), [1, 128]])
                        nc.tensor.matmul(ps_t[:, i * k:(i + 1) * k],
                                         lhsT, ones_ap)

                # evict PSUM -> SBUF (inv is folded into the ones weights),
                # out DMA issued from the mostly-idle ACT engine so its
                # descriptor generation never blocks the sh stream on SP
                osb = osbp.tile([128, k * nb], mybir.dt.float32, tag="osb")
                nc.scalar.copy(osb[:], ps_t[:])
                dst = bass.AP(out.ap().tensor, ch["out_off"],
                              [[tot["out"], 128], [1, k * nb]])
                nc.scalar.dma_start(dst, osb[:])

    nc.compile()
    _NC_CACHE[key] = nc
    return nc


# ---------------------------------------------------------------- host shard
def shard_inputs(sh_vectors, cutoffs, receivers, inv_avg_num_neighbors):
    sh_np = np.ascontiguousarray(np.asarray(sh_vectors, dtype=np.float32))
    cut_np = np.asarray(cutoffs, dtype=np.float32).ravel()
    rec = np.asarray(receivers).astype(np.int64).ravel()
    inv_val = np.float32(np.asarray(inv_avg_num_neighbors).ravel()[0])

    order = np.argsort(rec, kind="stable")
    rec_sorted = rec[order]
    first = np.searchsorted(rec_sorted, rec_sorted, side="left")
    occ = (np.arange(rec.size) - first).astype(np.int64)
    bounds = np.searchsorted(rec_sorted, np.arange(0, N_NODES + 1, NPC))

    deg = np.bincount(rec, minlength=N_NODES).reshape(N_CORES, NPC)
    node_orders, pos_of_node, dsort = [], [], np.empty((N_CORES, NPC), np.int64)
    for c in range(N_CORES):
        no = np.argsort(-deg[c], kind="stable")
        node_orders.append(no)
        pon = np.empty(NPC, dtype=np.int64)
        pon[no] = np.arange(NPC)
        pos_of_node.append(pon)
        dsort[c] = deg[c][no]
    dmax = dsort.max(axis=0)

    chunks, tot, sched = plan_chunks(dmax)

    # per-chunk lookup arrays indexed by chunk id
    nch = len(chunks)
    g_start = np.array([ch["node_start"] for ch in chunks], np.int64)
    g_k = np.array([ch["k"] for ch in chunks], np.int64)
    g_c = np.array([ch["c"] for ch in chunks], np.int64)
    g_F = np.array([ch["F"] for ch in chunks], np.int64)
    g_sh = np.array([ch["sh_off"] for ch in chunks], np.int64)
    g_cut = np.array([ch["cut_off"] for ch in chunks], np.int64)
    g_out = np.array([ch["out_off"] for ch in chunks], np.int64)
    g_end = np.concatenate([g_start[1:], [tot["nodes"]]])

    # static ones blocks with the inv scale folded into the weights (the PE
    # matmul then produces the final scaled sums directly)
    ones_dev = np.zeros((128, tot["ones"]), dtype=ml_dtypes.bfloat16)
    for ch in chunks:
        k, c = ch["k"], ch["c"]
        p = np.arange(k * c)
        ones_dev[p, ch["ones_off"] + p // c] = inv_val

    sh_bf = sh_np.astype(ml_dtypes.bfloat16)
    cut_bf = cut_np.astype(ml_dtypes.bfloat16)

    in_maps = []
    decode = []   # per core arrays to invert the layout on output
    for cid in range(N_CORES):
        lo, hi = bounds[cid], bounds[cid + 1]
        e = order[lo:hi]
        l = rec_sorted[lo:hi] - cid * NPC
        o = occ[lo:hi]
        q = pos_of_node[cid][l]

        gi = np.searchsorted(g_start, q, side="right") - 1
        k_e, c_e, F_e = g_k[gi], g_c[gi], g_F[gi]
        dq = q - g_start[gi]
        jj = dq // k_e
        n_lo = dq - jj * k_e
        i_loc = jj >> 3
        n8 = jj & 7
        p = n_lo * c_e + o
        assert (o < c_e).all(), "slot overflow: degree exceeds group capacity"

        sh_dev = np.zeros(tot["sh"], dtype=ml_dtypes.bfloat16)
        base = g_sh[gi] + p * F_e + 128 * i_loc + n8
        shv = sh_bf[e]
        for d in range(D_SH):
            sh_dev[base + 8 * d] = shv[:, d]
        cut_dev = np.zeros((128, tot["cut"]), dtype=ml_dtypes.bfloat16)
        cut_dev[p, g_cut[gi] + jj] = cut_bf[e]
        in_maps.append({"sh": sh_dev, "cut": cut_dev, "ones": ones_dev})

        # output decode for every position q in [0, NPC)
        qq = np.arange(NPC, dtype=np.int64)
        gq = np.searchsorted(g_start, qq, side="right") - 1
        dqq = qq - g_start[gq]
        jjq = dqq // g_k[gq]
        col = g_out[gq] + (jjq >> 3) * g_k[gq] + (dqq - jjq * g_k[gq])
        row8 = jjq & 7
        decode.append((col, row8))

    return in_maps, chunks, tot, sched, node_orders, decode


# ---------------------------------------------------------------- profiling
def _install_ntff_shim() -> bool:
    """This image's antenv lacks the axon_hooks shim that bass_utils imports
    for trace=True under axon.  Recreate it from trn_agent_boot's ctypes hook
    so NTFF profiling works.  Returns True on success."""
    try:
        import sys
        import types

        import antenv

        if getattr(antenv, "axon_hooks", None) is not None:
            return True
        import trn_agent_boot.trn_boot as tb

        hook = tb._ntff_profile_via_ctypes("/opt/axon/libaxon_pjrt.so")
        mod = types.ModuleType("antenv.axon_hooks")
        mod._hook = hook
        mod.get_axon_ntff_profile_hook = lambda: mod._hook
        mod.set_axon_ntff_profile_hook = lambda h: setattr(mod, "_hook", h)
        sys.modules["antenv.axon_hooks"] = mod
        antenv.axon_hooks = mod
        return hook is not None
    except Exception as e:  # profiling is best-effort; the run must not break
        print(f"ntff shim unavailable: {e!r}")
        return False


# ---------------------------------------------------------------- entrypoint
def kernel(sh_vectors, cutoffs, receivers, inv_avg_num_neighbors) -> np.ndarray:
    global LAST_RESULTS
    from concourse.bass_utils import run_bass_kernel_spmd

    in_maps, chunks, tot, sched, node_orders, decode = shard_inputs(
        sh_vectors, cutoffs, receivers, inv_avg_num_neighbors)
    key = tuple((ch["k"], ch["c"], ch["nb"]) for ch in chunks)
    nc = build_nc(key, chunks, tot, sched)

    trace = os.environ.get("KERNEL_TRACE", "0") == "1"
    if trace:
        trace = _install_ntff_shim()
    res = run_bass_kernel_spmd(nc, in_maps, core_ids=list(range(N_CORES)),
                               trace=trace)
    LAST_RESULTS = res

    full = np.empty((N_NODES, D_SH), dtype=np.float32)
    for cid in range(N_CORES):
        o = np.asarray(res.results[cid]["out"], dtype=np.float32)
        col, row8 = decode[cid]
        blk = np.empty((NPC, D_SH), dtype=np.float32)
        for d in range(D_SH):
            blk[:, d] = o[8 * d + row8, col]
        full[cid * NPC + node_orders[cid]] = blk
    return full


# revision 16
# speedup vs baseline: 1.2291x; 1.1365x over previous
"""Trainium2 kernel for nn_EuclideanEmbedding (edge-scale + segment_sum).

Computes: out[n, :] = inv * sum_{e: receivers[e]==n} sh_vectors[e, :] * cutoffs[e]

Distribution (host side, inside kernel()):
  - Edges sharded across the 8 NeuronCores BY RECEIVER NODE RANGE: core c owns
    nodes [c*6250, (c+1)*6250) and exactly the edges targeting them.  Each core
    produces a disjoint output slice, so no collective is needed.

Device layout (v2 — bf16 + PE-array segmented reduction):
  - Nodes are degree-sorted (desc) and packed into capacity groups.  A group
    with slot capacity c stacks k = floor(128/c) node-columns on the partition
    axis: partition p = n_lo*c + s  (n_lo in [0,k), slot s in [0,c)).
    Free axis per node-column block of 8 nodes: col = 128*i + d*8 + n8.
  - sh and cutoffs are converted to bf16 on the host (harness gate is
    rel_err < 2e-2; bf16 keeps us ~5e-3).  This halves HBM traffic — the
    fp32 baseline was HBM-bound at ~64us of DMA window.
  - The elementwise multiply (sh * cut) runs as bf16 TensorTensor ops split
    between the Vector and GpSimd engines (bf16 packed innermost -> DVE 2x
    mode).  The cut operand broadcasts over d via a zero-stride AP dim.
  - The segmented reduction runs on the OTHERWISE-IDLE PE array: one matmul
    per 8-node block with lhsT (stationary) = scaled data [kc, 128] and
    rhs (moving) = a static ones-block [kc, k] (ones[p, j] = (p // c == j)).
    out[d*8+n8, j] = sum_s scl[(j,s), d*8+n8] lands in PSUM fp32.  This
    removes the ~30us dtype-independent DVE tensor_reduce of the baseline.
  - PSUM eviction via the Scalar (ACT) engine activation-copy, which also
    applies the inv_avg_num_neighbors scale for free.
  - Output written per chunk (overlapped), not as one serial tail DMA.
"""

import os

import ml_dtypes
import numpy as np

# ---------------------------------------------------------------- constants
N_NODES = 50_000
D_SH = 16
N_CORES = 8
NPC = N_NODES // N_CORES          # 6250 nodes per core
NB_MAX = 28                       # max 8-node blocks per chunk (DMA/psum unit)
# The multiply runs entirely on the Vector engine (bf16 2x mode, ~0.62 ns/col
# measured).  GpSimd/Pool measured 2.5-4.9 ns/col on this AP shape and its
# slow ops became the pipeline critical path, so it gets no share.

_NC_CACHE: dict = {}
LAST_RESULTS = None  # BassKernelResults of the most recent run (for test.py)


# ---------------------------------------------------------------- geometry
def plan_chunks(dmax: np.ndarray):
    """Greedy capacity grouping from the SPMD-uniform per-position max degree
    (descending).  Returns a list of chunk dicts with all offsets."""
    q, groups = 0, []
    npos = dmax.shape[0]
    while q < npos:
        d0 = max(int(dmax[q]), 1)
        assert d0 <= 128, f"node degree {d0} > 128 unsupported by this layout"
        k = max(1, 128 // d0)
        q2 = q
        while q2 < npos and max(1, 128 // max(int(dmax[q2]), 1)) == k:
            q2 += 1
        n = -(-(q2 - q) // (8 * k)) * (8 * k)
        n = min(n, -(-(npos - q) // (8 * k)) * (8 * k))
        c = int(dmax[q:min(q + n, npos)].max())
        groups.append((k, max(c, 1), n))
        q += n

    chunks = []
    node_start = sh_off = cut_off = out_off = ones_off = 0
    for k, c, n in groups:
        nb_total = n // (8 * k)
        done = 0
        while done < nb_total:
            nb = min(NB_MAX, nb_total - done)
            kc = k * c
            F = 128 * nb
            chunks.append(dict(k=k, c=c, kc=kc, nb=nb, F=F,
                               node_start=node_start, sh_off=sh_off,
                               cut_off=cut_off, out_off=out_off,
                               ones_off=ones_off))
            node_start += 8 * k * nb
            sh_off += kc * F
            cut_off += 8 * nb
            out_off += k * nb
            ones_off += k
            done += nb
    tot = dict(nodes=node_start, sh=sh_off, cut=cut_off, out=out_off,
               ones=ones_off)
    # device-side processing order: smallest chunk first (fast pipeline
    # priming), then the rest descending so another small chunk lands last
    # (short drain tail); host layout offsets above are order-independent.
    by_size = sorted(range(len(chunks)),
                     key=lambda i: chunks[i]["kc"] * chunks[i]["F"])
    sched = by_size[:1] + by_size[1:][::-1]
    return chunks, tot, sched


# ---------------------------------------------------------------- device IR
def build_nc(key, chunks, tot, sched):
    if key in _NC_CACHE:
        return _NC_CACHE[key]

    import concourse.bacc as bacc
    import concourse.bass as bass
    import concourse.mybir as mybir
    from concourse import tile

    nc = bacc.Bacc("TRN2", target_bir_lowering=False, debug=False)

    sh = nc.dram_tensor("sh", [tot["sh"]], mybir.dt.bfloat16,
                        kind="ExternalInput")
    cut = nc.dram_tensor("cut", [128, tot["cut"]], mybir.dt.bfloat16,
                         kind="ExternalInput")
    ones = nc.dram_tensor("ones", [128, tot["ones"]], mybir.dt.bfloat16,
                          kind="ExternalInput")
    out = nc.dram_tensor("out", [128, tot["out"]], mybir.dt.float32,
                         kind="ExternalOutput")

    with tile.TileContext(nc) as tc:
        with (
            tc.tile_pool(name="const", bufs=1) as cpool,
            tc.tile_pool(name="shp", bufs=5) as shp,
            tc.tile_pool(name="cutp", bufs=5) as cutp,
            tc.tile_pool(name="sclv", bufs=4) as sclvp,
            tc.tile_pool(name="osb", bufs=2) as osbp,
            tc.tile_pool(name="ps", bufs=4, space="PSUM") as psp,
        ):
            ones_t = cpool.tile([128, tot["ones"]], mybir.dt.bfloat16)

            for chi, ci in enumerate(sched):
                ch = chunks[ci]
                k, c, kc, nb, F = (ch["k"], ch["c"], ch["kc"], ch["nb"],
                                   ch["F"])
                # per-chunk cut slice, then the bulk sh data
                cut_t = cutp.tile([kc, 8 * nb], mybir.dt.bfloat16, tag="cut")
                csrc = bass.AP(cut.ap().tensor, ch["cut_off"],
                               [[tot["cut"], kc], [1, 8 * nb]])
                nc.sync.dma_start(cut_t[:], csrc)
                sh_t = shp.tile([kc, F], mybir.dt.bfloat16, tag="sh")
                src = bass.AP(sh.ap().tensor, ch["sh_off"], [[F, kc], [1, F]])
                nc.sync.dma_start(sh_t[:], src)
                if chi == 0:
                    # emit after the first sh chunk: ones is only needed by
                    # the first matmul, sh0+cut0 gate the first multiply
                    nc.sync.dma_start(ones_t[:], ones[:])

                # scl[p, i, d, n8] = sh[p, i, d, n8] * cut[p, 8*i + n8]
                # on Vector, in two halves so PE can start on the first half
                # while the second is still multiplying
                nbh = (nb + 1) // 2
                scl_tiles = []
                for b0, b1, tg in ((0, nbh, "scla"), (nbh, nb, "sclb")):
                    nbe = b1 - b0
                    if nbe <= 0:
                        continue
                    scl = sclvp.tile([kc, nbe * 128], mybir.dt.bfloat16,
                                     tag=tg)
                    pdim = list(sh_t[:].ap[0])
                    sh4 = bass.AP(sh_t[:].tensor, sh_t[:].offset + b0 * 128,
                                  [pdim, [128, nbe], [8, D_SH], [1, 8]])
                    scl4 = bass.AP(scl[:].tensor, scl[:].offset,
                                   [list(scl[:].ap[0]), [128, nbe],
                                    [8, D_SH], [1, 8]])
                    cut4 = bass.AP(cut_t[:].tensor,
                                   cut_t[:].offset + b0 * 8,
                                   [list(cut_t[:].ap[0]), [8, nbe],
                                    [0, D_SH], [1, 8]])
                    nc.vector.tensor_mul(scl4, sh4, cut4)
                    scl_tiles.append((scl, b0, b1))

                # PE: per 8-node block, out[(d,n8), j] = sum_s scl[(j,s),...]
                ps_t = psp.tile([128, k * nb], mybir.dt.float32, tag="ps")
                ones_ap = bass.AP(ones_t[:].tensor,
                                  ones_t[:].offset + ch["ones_off"],
                                  [[ones_t[:].ap[0][0], kc], [1, k]])
                for scl, b0, b1 in scl_tiles:
                    for i in range(b0, b1):
                        lhsT = bass.AP(scl[:].tensor,
                                       scl[:].offset + (i - b0) * 128,
                                       [list(scl[:].ap[0]), [1, 128]])
                        nc.tensor.matmul(ps_t[:, i * k:(i + 1) * k],
                                         lhsT, ones_ap)

                # evict PSUM -> SBUF (inv is folded into the ones weights),
                # out DMA issued from the mostly-idle ACT engine so its
                # descriptor generation never blocks the sh stream on SP
                osb = osbp.tile([128, k * nb], mybir.dt.float32, tag="osb")
                nc.scalar.copy(osb[:], ps_t[:])
                dst = bass.AP(out.ap().tensor, ch["out_off"],
                              [[tot["out"], 128], [1, k * nb]])
                nc.scalar.dma_start(dst, osb[:])

    nc.compile()
    _NC_CACHE[key] = nc
    return nc


# ---------------------------------------------------------------- host shard
def shard_inputs(sh_vectors, cutoffs, receivers, inv_avg_num_neighbors):
    sh_np = np.ascontiguousarray(np.asarray(sh_vectors, dtype=np.float32))
    cut_np = np.asarray(cutoffs, dtype=np.float32).ravel()
    rec = np.asarray(receivers).astype(np.int64).ravel()
    inv_val = np.float32(np.asarray(inv_avg_num_neighbors).ravel()[0])

    order = np.argsort(rec, kind="stable")
    rec_sorted = rec[order]
    first = np.searchsorted(rec_sorted, rec_sorted, side="left")
    occ = (np.arange(rec.size) - first).astype(np.int64)
    bounds = np.searchsorted(rec_sorted, np.arange(0, N_NODES + 1, NPC))

    deg = np.bincount(rec, minlength=N_NODES).reshape(N_CORES, NPC)
    node_orders, pos_of_node, dsort = [], [], np.empty((N_CORES, NPC), np.int64)
    for c in range(N_CORES):
        no = np.argsort(-deg[c], kind="stable")
        node_orders.append(no)
        pon = np.empty(NPC, dtype=np.int64)
        pon[no] = np.arange(NPC)
        pos_of_node.append(pon)
        dsort[c] = deg[c][no]
    dmax = dsort.max(axis=0)

    chunks, tot, sched = plan_chunks(dmax)

    # per-chunk lookup arrays indexed by chunk id
    nch = len(chunks)
    g_start = np.array([ch["node_start"] for ch in chunks], np.int64)
    g_k = np.array([ch["k"] for ch in chunks], np.int64)
    g_c = np.array([ch["c"] for ch in chunks], np.int64)
    g_F = np.array([ch["F"] for ch in chunks], np.int64)
    g_sh = np.array([ch["sh_off"] for ch in chunks], np.int64)
    g_cut = np.array([ch["cut_off"] for ch in chunks], np.int64)
    g_out = np.array([ch["out_off"] for ch in chunks], np.int64)
    g_end = np.concatenate([g_start[1:], [tot["nodes"]]])

    # static ones blocks with the inv scale folded into the weights (the PE
    # matmul then produces the final scaled sums directly)
    ones_dev = np.zeros((128, tot["ones"]), dtype=ml_dtypes.bfloat16)
    for ch in chunks:
        k, c = ch["k"], ch["c"]
        p = np.arange(k * c)
        ones_dev[p, ch["ones_off"] + p // c] = inv_val

    sh_bf = sh_np.astype(ml_dtypes.bfloat16)
    cut_bf = cut_np.astype(ml_dtypes.bfloat16)

    in_maps = []
    decode = []   # per core arrays to invert the layout on output
    for cid in range(N_CORES):
        lo, hi = bounds[cid], bounds[cid + 1]
        e = order[lo:hi]
        l = rec_sorted[lo:hi] - cid * NPC
        o = occ[lo:hi]
        q = pos_of_node[cid][l]

        gi = np.searchsorted(g_start, q, side="right") - 1
        k_e, c_e, F_e = g_k[gi], g_c[gi], g_F[gi]
        dq = q - g_start[gi]
        jj = dq // k_e
        n_lo = dq - jj * k_e
        i_loc = jj >> 3
        n8 = jj & 7
        p = n_lo * c_e + o
        assert (o < c_e).all(), "slot overflow: degree exceeds group capacity"

        sh_dev = np.zeros(tot["sh"], dtype=ml_dtypes.bfloat16)
        base = g_sh[gi] + p * F_e + 128 * i_loc + n8
        shv = sh_bf[e]
        for d in range(D_SH):
            sh_dev[base + 8 * d] = shv[:, d]
        cut_dev = np.zeros((128, tot["cut"]), dtype=ml_dtypes.bfloat16)
        cut_dev[p, g_cut[gi] + jj] = cut_bf[e]
        in_maps.append({"sh": sh_dev, "cut": cut_dev, "ones": ones_dev})

        # output decode for every position q in [0, NPC)
        qq = np.arange(NPC, dtype=np.int64)
        gq = np.searchsorted(g_start, qq, side="right") - 1
        dqq = qq - g_start[gq]
        jjq = dqq // g_k[gq]
        col = g_out[gq] + (jjq >> 3) * g_k[gq] + (dqq - jjq * g_k[gq])
        row8 = jjq & 7
        decode.append((col, row8))

    return in_maps, chunks, tot, sched, node_orders, decode


# ---------------------------------------------------------------- profiling
def _install_ntff_shim() -> bool:
    """This image's antenv lacks the axon_hooks shim that bass_utils imports
    for trace=True under axon.  Recreate it from trn_agent_boot's ctypes hook
    so NTFF profiling works.  Returns True on success."""
    try:
        import sys
        import types

        import antenv

        if getattr(antenv, "axon_hooks", None) is not None:
            return True
        import trn_agent_boot.trn_boot as tb

        hook = tb._ntff_profile_via_ctypes("/opt/axon/libaxon_pjrt.so")
        mod = types.ModuleType("antenv.axon_hooks")
        mod._hook = hook
        mod.get_axon_ntff_profile_hook = lambda: mod._hook
        mod.set_axon_ntff_profile_hook = lambda h: setattr(mod, "_hook", h)
        sys.modules["antenv.axon_hooks"] = mod
        antenv.axon_hooks = mod
        return hook is not None
    except Exception as e:  # profiling is best-effort; the run must not break
        print(f"ntff shim unavailable: {e!r}")
        return False


# ---------------------------------------------------------------- entrypoint
def kernel(sh_vectors, cutoffs, receivers, inv_avg_num_neighbors) -> np.ndarray:
    global LAST_RESULTS
    from concourse.bass_utils import run_bass_kernel_spmd

    in_maps, chunks, tot, sched, node_orders, decode = shard_inputs(
        sh_vectors, cutoffs, receivers, inv_avg_num_neighbors)
    key = tuple((ch["k"], ch["c"], ch["nb"]) for ch in chunks)
    nc = build_nc(key, chunks, tot, sched)

    trace = os.environ.get("KERNEL_TRACE", "0") == "1"
    if trace:
        trace = _install_ntff_shim()
    res = run_bass_kernel_spmd(nc, in_maps, core_ids=list(range(N_CORES)),
                               trace=trace)
    LAST_RESULTS = res

    full = np.empty((N_NODES, D_SH), dtype=np.float32)
    for cid in range(N_CORES):
        o = np.asarray(res.results[cid]["out"], dtype=np.float32)
        col, row8 = decode[cid]
        blk = np.empty((NPC, D_SH), dtype=np.float32)
        for d in range(D_SH):
            blk[:, d] = o[8 * d + row8, col]
        full[cid * NPC + node_orders[cid]] = blk
    return full
